# revision 45
# baseline (speedup 1.0000x reference)
"""Causal self-attention (K/Q swapped variant) on 8 trn2 NeuronCores.

Sharding: core c = (b, g) with b = c // 4 (batch), g = c % 4 (head group of
4 heads).  Each core computes, for its batch and heads, the full attention
and a partial output projection (its heads' rows of Wproj); the host sums
the 4 partials per batch and adds bproj.

v3 design notes (driven by perfetto traces):
  - exp fused across the head pair: both hh score tiles land in one
    [128, 1024] PSUM tile (2 banks), one EXP instruction per (pair, mblock).
  - reciprocal -> reciprocal_approx_fast on the full [128,512] accumulator
    (custom-DVE ops are full-tile only; rows != 64 are don't-care lanes).
  - v-bias folded into the V projection (softmax rows sum to 1).
  - causal mask stored once and broadcast across the hh dim via a 0-stride
    free-dim AP.
  - PSUM plan: s-ring 2x[128,1024] (4 banks) + per-head O accumulators
    oh0/oh1 (2 banks, pairs processed sequentially; proj reuses these) +
    kqv/transpose ring (2 banks) = 8 banks exactly.
  - startup DMA consolidated: DMA cost is ~42ns per descriptor and one
    descriptor per partition row per dma_start, so inputs are repacked on
    the host into a few fat [128, wide] tensors (xT nb-major so the first
    n-block arrives in a single transfer).
  - output partials in bf16 (halves output DMA).
"""

import os
import sys

if "/opt/trn_rl_repo" not in sys.path:
    sys.path.insert(0, "/opt/trn_rl_repo")

import numpy as np

B, N, D, H = 2, 2048, 1024, 16
DK = 64
NCORES = 8
GROUPS = 4          # head groups
HPC = H // GROUPS   # 4 heads per core
CH = D // 128       # 8 contraction chunks
NB = N // 512       # 4 n-blocks
MBS = N // 128      # 16 m-blocks

_CACHE = {}


def _build_program():
    import concourse.bacc as bacc
    import concourse.mybir as mybir
    from concourse.tile import TileContext
    from contextlib import ExitStack

    f32 = mybir.dt.float32
    bf = mybir.dt.bfloat16
    EXP = mybir.ActivationFunctionType.Exp

    nc = bacc.Bacc(
        "TRN2",
        target_bir_lowering=False,
        debug=False,
        enable_asserts=False,
        num_devices=NCORES,
    )

    # xTr: col = nb*4096 + c*512 + n  (nb-major so block 0 is one transfer)
    xTr = nc.dram_tensor("xTr", [128, NB * CH * 512], bf, kind="ExternalInput").ap()
    wk = nc.dram_tensor("wk", [128, CH * 256], bf, kind="ExternalInput").ap()
    wq = nc.dram_tensor("wq", [128, CH * 256], bf, kind="ExternalInput").ap()
    wv = nc.dram_tensor("wv", [128, CH * 256], bf, kind="ExternalInput").ap()
    wp = nc.dram_tensor("wp", [128, 2 * D], bf, kind="ExternalInput").ap()
    masks = nc.dram_tensor("masks", [128, 4 * 512], bf, kind="ExternalInput").ap()
    identones = nc.dram_tensor("identones", [128, 320], bf, kind="ExternalInput").ap()
    bias = nc.dram_tensor("bias", [128, 6], f32, kind="ExternalInput").ap()
    out_p = nc.dram_tensor("out_p", [N, D], bf, kind="ExternalOutput").ap()

    with TileContext(nc) as tc, ExitStack() as ctx:
        constp = ctx.enter_context(tc.tile_pool(name="const", bufs=1))
        storep = ctx.enter_context(tc.tile_pool(name="store", bufs=1))
        ep = ctx.enter_context(tc.tile_pool(name="e", bufs=12))
        onnp = ctx.enter_context(tc.tile_pool(name="onn", bufs=2))
        rcp = ctx.enter_context(tc.tile_pool(name="rc", bufs=2))
        osp = ctx.enter_context(tc.tile_pool(name="os", bufs=3))
        # PSUM: kq ring 2x[128,512]f32 = 2 banks (KQV groups + V-transpose
        # staging ONLY -- so the next block's KQV never waits on proj or
        # normalize), s ring 2x[128,1024]f32 = 4 banks (scores only),
        # oh0/oh1 [128,512]f32 = 2 banks (O accumulators + bc broadcasts +
        # proj tiles: each of those allocations only waits on reads that
        # are true dependencies of its own chain).  Total 8.
        kqp = ctx.enter_context(tc.tile_pool(name="kq", bufs=2, space="PSUM"))
        sp = ctx.enter_context(tc.tile_pool(name="s", bufs=2, space="PSUM"))
        op = ctx.enter_context(tc.tile_pool(name="o", bufs=1, space="PSUM"))

        # ---- big fat input DMAs (few descriptors, wide rows) ----
        xt_all = constp.tile([128, NB * CH * 512], bf, tag="xt")
        nc.sync.dma_start(xt_all[:, 0:2048], xTr[:, 0:2048])
        nc.sync.dma_start(xt_all[:, 2048:4096], xTr[:, 2048:4096])
        wk_sb = constp.tile([128, CH * 256], bf, tag="wk")
        wq_sb = constp.tile([128, CH * 256], bf, tag="wq")
        wv_sb = constp.tile([128, CH * 256], bf, tag="wv")
        nc.sync.dma_start(wk_sb[:], wk[:, :])
        nc.sync.dma_start(wq_sb[:], wq[:, :])
        nc.sync.dma_start(wv_sb[:], wv[:, :])
        for nb in range(1, NB):
            nc.sync.dma_start(
                xt_all[:, nb * 4096:(nb + 1) * 4096], xTr[:, nb * 4096:(nb + 1) * 4096]
            )
        io_sb = constp.tile([128, 320], bf, tag="io")
        nc.sync.dma_start(io_sb[:], identones[:, :])
        bias_sb = constp.tile([128, 6], f32, tag="bias")
        nc.sync.dma_start(bias_sb[:], bias[:, :])
        masks_sb = constp.tile([128, 4 * 512], bf, tag="masks")
        nc.sync.dma_start(masks_sb[:], masks[:, :])
        wp_sb = constp.tile([128, 2 * D], bf, tag="wp")
        nc.sync.dma_start(wp_sb[:], wp[:, :])

        ident_sb = io_sb[:, 0:128]
        ones_sb = io_sb[:, 128:192]
        # row 0, cols 192:320: [0]*64 + [1]*64 -- broadcasts the hh=1
        # reciprocal row into output partitions 64..127
        ones_hi = io_sb[0:1, 192:320]

        # scratch for PE warm-up matmuls (memset, so no DMA dependency)
        scr_sb = constp.tile([128, 512], bf, tag="scr")
        nc.gpsimd.memset(scr_sb[:], 0.0)

        def xt(nb, c):
            return xt_all[:, nb * 4096 + c * 512: nb * 4096 + (c + 1) * 512]

        # ---- persistent activation storage ----
        kt = storep.tile([128, 2 * N], bf, tag="kt")    # [pairfeat, pair*N + n]
        qt = storep.tile([128, 2 * N], bf, tag="qt")
        # even heads (hh=0): 65-wide blocks [dk(64) | ones] -> O in PSUM
        # rows 0..64 (denominator row 64).  odd heads (hh=1): 128-wide
        # blocks [ones | zeros | dk@64..127] -> O lands directly in PSUM
        # rows 64..127 (denominator row 0), so the normalized output
        # writes otp partitions 64..127 with no partition-shift DMA.
        v_sb = [storep.tile(
                    [128, MBS * (65 if h % 2 == 0 else 128)], bf,
                    tag=f"v{h}", name=f"v{h}")
                for h in range(HPC)]
        otp = [storep.tile([128, N], bf, tag=f"otp{p}", name=f"otp{p}")
               for p in range(2)]
        for h in range(HPC):
            if h % 2 == 0:
                nc.vector.tensor_copy(
                    v_sb[h].rearrange("p (m c) -> p m c", c=65)[:, :, 64],
                    ones_sb[:, 0:16],
                )
            else:
                nc.gpsimd.memset(v_sb[h][:], 0.0)
                nc.vector.tensor_copy(
                    v_sb[h].rearrange("p (m c) -> p m c", c=128)[:, :, 0],
                    ones_sb[:, 0:16],
                )

        masks_r = masks_sb.rearrange("p (r n) -> p r n", r=4)

        # ---- KQV emission helpers -------------------------------------
        # The kq ring (bufs=2) paces the pipeline: because the bc tiles of
        # block j's normalize sit between block j+1's K/Q groups in the
        # ring sequence, block j+1's KQV physically cannot run before
        # block j's attention has made matching progress.  Without this,
        # the scheduler (greedy, work-conserving) front-loads all KQV
        # work into the early blocks and block 3's ACT-bound stretch runs
        # with an empty PE.
        def emit_kq_pair(nb, pair, warm=False):
            for wsb, half in ((wk_sb, 0), (wq_sb, 1)):
                ps = kqp.tile(
                    [128, 512], f32, tag="kq", name=f"kq{nb}p{pair}h{half}"
                )
                if warm and half == 0:
                    # warm-up: keep the PE busy through the initial
                    # input-DMA wait so HAM unthrottles to 2.4 GHz before
                    # real work; the first real K matmul's start=True
                    # clears has_written, so these results are discarded.
                    for w in range(48):
                        nc.tensor.matmul(
                            ps[:],
                            scr_sb[:, 0:128],
                            scr_sb[:],
                            start=(w == 0),
                            stop=False,
                        )
                for c in range(CH):
                    nc.tensor.matmul(
                        ps[:],
                        wsb[:, c * 256 + pair * 128: c * 256 + (pair + 1) * 128],
                        xt(nb, c),
                        start=(c == 0),
                        stop=(c == CH - 1),
                    )
                dst = kt if half == 0 else qt
                nc.vector.tensor_scalar_add(
                    dst[:, pair * N + nb * 512: pair * N + (nb + 1) * 512],
                    ps[:],
                    bias_sb[:, 2 * half + pair:2 * half + pair + 1],
                )

        def emit_v(nb):
            # V^T projection (one 1-bank group per pair), bias folded in
            vt = constp.tile([128, 1024], bf, tag="vt", name=f"vt{nb}", bufs=2)
            for pair in range(2):
                psv = kqp.tile([128, 512], f32, tag="kq", name=f"psv{nb}p{pair}")
                for c in range(CH):
                    nc.tensor.matmul(
                        psv[:],
                        wv_sb[:, c * 256 + pair * 128: c * 256 + (pair + 1) * 128],
                        xt(nb, c),
                        start=(c == 0),
                        stop=(c == CH - 1),
                    )
                nc.vector.tensor_scalar_add(
                    vt[:, pair * 512:(pair + 1) * 512],
                    psv[:],
                    bias_sb[:, 4 + pair:5 + pair],
                )
            # transpose V^T -> per-head [m, 64] blocks
            pst = kqp.tile([128, 1024], bf, tag="kq", name=f"pst{nb}")
            for pair in range(2):
                for sub in range(4):
                    k8 = pair * 4 + sub
                    nc.tensor.transpose(
                        pst[:, k8 * 128:(k8 + 1) * 128],
                        vt[:, pair * 512 + sub * 128: pair * 512 + (sub + 1) * 128],
                        ident_sb,
                    )
                    mb = nb * 4 + sub
                    nc.vector.tensor_copy(
                        v_sb[2 * pair][:, mb * 65: mb * 65 + 64],
                        pst[:, k8 * 128: k8 * 128 + 64],
                    )
                    nc.vector.tensor_copy(
                        v_sb[2 * pair + 1][:, mb * 128 + 64: mb * 128 + 128],
                        pst[:, k8 * 128 + 64: k8 * 128 + 128],
                    )

        # ---- software pipeline: KQV(0) up front, then per block j:
        # attention(j) with KQV(j+1) emitted at paced points inside it ----
        emit_kq_pair(0, 0, warm=True)
        emit_kq_pair(0, 1)
        emit_v(0)

        for j in range(NB):
            nm = 4 * j + 4
            for pair in range(2):
                o_ps = {}
                for hh in range(2):
                    # rows 0-64 hold O_aug; full-tile alloc so the custom-DVE
                    # recip (subdim=False) can read the whole tile.
                    o_ps[hh] = op.tile(
                        [128, 512], f32, tag=f"oh{hh}", name=f"o{j}{pair}{hh}"
                    )
                for mb in range(nm):
                    rdiag = mb - 4 * j
                    c0 = 128 * rdiag if rdiag > 0 else 0
                    s = sp.tile([128, 1024], f32, tag="s")
                    for hh in range(2):
                        base = hh * 64
                        nc.tensor.matmul(
                            s[:, hh * 512 + c0:(hh + 1) * 512],
                            qt[base:base + 64,
                               pair * N + mb * 128: pair * N + (mb + 1) * 128],
                            kt[base:base + 64,
                               pair * N + j * 512 + c0: pair * N + (j + 1) * 512],
                        )
                    e = ep.tile([128, 1024], bf, tag="e")
                    s3 = s.rearrange("p (h n) -> p h n", h=2)
                    e3 = e.rearrange("p (h n) -> p h n", h=2)
                    nc.scalar.activation(
                        e3[:, :, c0:512], s3[:, :, c0:512], EXP, scale=0.125
                    )
                    if rdiag >= 0:
                        m_b = (
                            masks_r[:, rdiag, c0:512]
                            .rearrange("p (o n) -> p o n", o=1)
                            .broadcast_to([128, 2, 512 - c0])
                        )
                        nc.vector.tensor_mul(
                            e3[:, :, c0:512], e3[:, :, c0:512], m_b
                        )
                    nc.tensor.matmul(
                        o_ps[0][0:65, c0:512],
                        v_sb[2 * pair][:, mb * 65: mb * 65 + 65],
                        e[:, c0:512],
                        start=(mb == 0),
                        stop=(mb == nm - 1),
                    )
                    nc.tensor.matmul(
                        o_ps[1][0:128, c0:512],
                        v_sb[2 * pair + 1][:, mb * 128: mb * 128 + 128],
                        e[:, 512 + c0:1024],
                        start=(mb == 0),
                        stop=(mb == nm - 1),
                    )

                # next block's K/Q for this pair: lands in the kq ring
                # BEFORE this pair's bc tiles, so it may overlap this
                # pair's attention but nothing earlier
                if j + 1 < NB:
                    emit_kq_pair(j + 1, pair)

                # ---- normalize tail for this pair (emitted immediately so
                # its recip runs as soon as the O accumulation completes and
                # the other pair's AV chain, which reuses the oh banks, is
                # not starved behind this pair's masks in the DVE queue) ----
                rcb = rcp.tile([128, 1024], bf, tag="rcb")
                onn = onnp.tile([128, 1024], bf, tag="onn")
                for hh in (1, 0):
                    # full-tile custom-DVE recip (other rows don't-care)
                    rc32 = rcp.tile([128, 512], f32, tag=f"rc32{hh}")
                    nc.vector.reciprocal_approx_fast(
                        out=rc32[:], in_=o_ps[hh][:]
                    )
                    # hh=0: denominator row 64 -> broadcast to rows 0..63;
                    # hh=1: denominator row 0 -> broadcast to rows 64..127
                    drow = 64 if hh == 0 else 0
                    nc.vector.tensor_copy(
                        rcb[drow:drow + 1, hh * 512:(hh + 1) * 512],
                        rc32[drow:drow + 1, :],
                    )
                    bc = kqp.tile(
                        [128, 512], f32, tag="kq", name=f"bc{j}p{pair}h{hh}"
                    )
                    if hh == 0:
                        nc.tensor.matmul(
                            bc[0:64, :],
                            ones_sb[64:65, 0:64],
                            rcb[64:65, 0:512],
                        )
                        nc.vector.tensor_copy(
                            onn[0:64, 0:512], o_ps[0][0:64, :]
                        )
                        nc.vector.tensor_mul(
                            otp[pair][0:64, j * 512:(j + 1) * 512],
                            onn[0:64, 0:512],
                            bc[0:64, :],
                        )
                    else:
                        nc.tensor.matmul(
                            bc[:, :],
                            ones_hi,
                            rcb[0:1, 512:1024],
                        )
                        nc.vector.tensor_copy(
                            onn[64:128, 512:1024], o_ps[1][64:128, :]
                        )
                        # multiply straight out of the PSUM broadcast tile
                        # (fp32 PSUM x bf16 SBUF -> bf16)
                        nc.vector.tensor_mul(
                            otp[pair][64:128, j * 512:(j + 1) * 512],
                            onn[64:128, 512:1024],
                            bc[64:128, :],
                        )

            # next block's V projection + transposes: ring-gated behind
            # this block's last bc, i.e. can start only once this block's
            # attention is done (its own diagonal m-blocks need it early
            # in the next attention, which is exactly when it lands)
            if j + 1 < NB:
                emit_v(j + 1)

            # ---- final projection for output rows of this n-block ----
            # (PE gap-filler while the next block's attention is ACT-bound)
            for sub in range(4):
                nbk = 4 * j + sub
                os_t = osp.tile([128, D], bf, tag="os")
                for cb in range(2):
                    # block 3: the kq ring is otherwise free, and splitting
                    # off the oh ring lets the p2=0 half of the contraction
                    # run during pair1's attention
                    if j == 3:
                        fp = kqp.tile(
                            [128, 512], f32, tag="kq", name=f"fp{j}s{sub}c{cb}"
                        )
                    else:
                        fp = op.tile(
                            [128, 512], f32, tag=f"oh{cb}", name=f"fp{j}s{sub}c{cb}"
                        )
                    for p2 in range(2):
                        nc.tensor.matmul(
                            fp[:],
                            otp[p2][:, nbk * 128:(nbk + 1) * 128],
                            wp_sb[:, p2 * D + cb * 512: p2 * D + (cb + 1) * 512],
                            start=(p2 == 0),
                            stop=(p2 == 1),
                        )
                    # spread PSUM evictions across the scalar and vector
                    # engines; scalar gets the idle tail after block 3
                    if j == 3 or cb == 0:
                        nc.scalar.copy(os_t[:, cb * 512:(cb + 1) * 512], fp[:])
                    else:
                        nc.vector.tensor_copy(os_t[:, cb * 512:(cb + 1) * 512], fp[:])
                    if j == 3:
                        # exposed drain: ship each half as soon as it lands
                        nc.sync.dma_start(
                            out_p[nbk * 128:(nbk + 1) * 128,
                                  cb * 512:(cb + 1) * 512],
                            os_t[:, cb * 512:(cb + 1) * 512],
                        )
                if j != 3:
                    nc.sync.dma_start(out_p[nbk * 128:(nbk + 1) * 128, :], os_t[:])

    nc.compile()
    return nc


def _get_program():
    if "nc" not in _CACHE:
        _CACHE["nc"] = _build_program()
    return _CACHE["nc"]


def _prep_in_maps(x, Wkqv, bkqv, Wproj, bproj):
    import ml_dtypes
    bf = ml_dtypes.bfloat16

    x = np.asarray(x, np.float32)
    Wkqv = np.asarray(Wkqv, np.float32)
    bkqv = np.asarray(bkqv, np.float32)
    Wproj = np.asarray(Wproj, np.float32)

    # de-interleave kqv columns: col 3d+0 -> k_d, 3d+1 -> q_d, 3d+2 -> v_d
    Wk = Wkqv[:, :, 0::3]  # [H, D, DK]
    Wq = Wkqv[:, :, 1::3]
    Wv = Wkqv[:, :, 2::3]
    bk = bkqv[:, 0::3]     # [H, DK]
    bq = bkqv[:, 1::3]
    bv = bkqv[:, 2::3]

    masks = np.zeros((128, 4, 512), np.float32)
    mm = np.arange(128)[:, None]
    nn = np.arange(512)[None, :]
    for rr in range(4):
        masks[:, rr, :] = (128 * rr + mm <= nn).astype(np.float32)
    masks = np.ascontiguousarray(masks.reshape(128, 2048)).astype(bf)
    identones = np.zeros((128, 320), np.float32)
    identones[:, 0:128] = np.eye(128, dtype=np.float32)
    identones[:, 128:192] = 1.0
    identones[0, 256:320] = 1.0   # ones_hi: [0]*64 + [1]*64 on row 0
    identones = identones.astype(bf)

    def wlayout(Wg):  # [4, D, DK] -> [128, CH*256] (pair-major columns)
        arr = Wg.reshape(2, 2, CH, 128, DK)          # [pair, hh, ch, p, f]
        return np.ascontiguousarray(
            arr.transpose(2, 3, 0, 1, 4).reshape(CH, 128, 256)
            .transpose(1, 0, 2).reshape(128, CH * 256).astype(bf)
        )

    group_maps = []
    for g in range(GROUPS):
        hs = slice(g * HPC, (g + 1) * HPC)
        bias_t = np.zeros((128, 6), np.float32)
        for pair in range(2):
            h0, h1 = g * HPC + 2 * pair, g * HPC + 2 * pair + 1
            bias_t[0:64, pair] = bk[h0]
            bias_t[64:128, pair] = bk[h1]
            bias_t[0:64, 2 + pair] = bq[h0]
            bias_t[64:128, 2 + pair] = bq[h1]
            bias_t[0:64, 4 + pair] = bv[h0]
            bias_t[64:128, 4 + pair] = bv[h1]
        wp_c = np.ascontiguousarray(
            Wproj[g * HPC * DK:(g + 1) * HPC * DK].reshape(2, 128, D)
            .transpose(1, 0, 2).reshape(128, 2 * D).astype(bf)
        )
        group_maps.append({
            "wk": wlayout(Wk[hs]),
            "wq": wlayout(Wq[hs]),
            "wv": wlayout(Wv[hs]),
            "wp": wp_c,
            "bias": bias_t,
            "masks": masks,
            "identones": identones,
        })

    # xTr: [128, nb*4096 + c*512 + n] = x[b][nb*512+n, c*128+p]
    xTrs = []
    for b in range(B):
        xT = x[b].T.astype(bf)                        # [D, N]
        xTr = (xT.reshape(CH, 128, NB, 512)
               .transpose(1, 2, 0, 3).reshape(128, NB * CH * 512))
        xTrs.append(np.ascontiguousarray(xTr))
    in_maps = []
    for c in range(NCORES):
        b, g = c // GROUPS, c % GROUPS
        m = dict(group_maps[g])
        m["xTr"] = xTrs[b]
        in_maps.append(m)
    return in_maps


def _run(inputs, trace=False):
    from concourse.bass_utils import run_bass_kernel_spmd

    nc = _get_program()
    in_maps = _prep_in_maps(
        inputs["x"], inputs["Wkqv"], inputs["bkqv"], inputs["Wproj"], inputs["bproj"]
    )
    res = run_bass_kernel_spmd(nc, in_maps, core_ids=list(range(NCORES)), trace=trace)
    bproj = np.asarray(inputs["bproj"], np.float32)
    out = np.empty((B, N, D), np.float32)
    for b in range(B):
        acc = res.results[b * GROUPS]["out_p"].astype(np.float32)
        for g in range(1, GROUPS):
            acc = acc + res.results[b * GROUPS + g]["out_p"].astype(np.float32)
        out[b] = acc + bproj[None, :]
    return out, res


def kernel(**inputs):
    return _run(inputs)[0]

